# revision 33
# baseline (speedup 1.0000x reference)
"""GAT (gnn_message_passing) Trainium2 Bass kernel — 8-core SPMD.

Contract: kernel(**inputs) -> np.ndarray with FULL inputs / FULL output.
Self-contained: hardcodes shapes; only imports the container's concourse stack.

Design:
- Stage A: per-core h = (x @ fb) @ Wg plus attention dots, written directly
  into one Shared-DRAM node table via indexed (indirect) DMA writes.
- A tiny barrier AllGather replaces the two big halo AllGathers.
- Edge phase: per 128-node destination tile, gather source rows (bf16,
  512B/row) + dst attention rows, compute edge softmax numerators on
  DVE/Act, accumulate per-node sums with indicator matmuls in PSUM,
  normalize, then fused 3-layer MLP + softmax.
"""
import sys

for _p in ("/opt/trn_rl_repo", "/root/.axon_site/_ro/trn_rl_repo"):
    if _p not in sys.path:
        sys.path.append(_p)

import numpy as np

# ---------------- problem constants (hardcoded per contract) ----------------
N = 50000
NF = 513
NFP = 640            # padded feature dim (5 * 128)
NMEL = 128
H, C = 4, 32
HC = H * C           # 128
E = 800000
NEG_ATT = 0.2
NEG_MLP = 0.01

NCORES = 8
TPC = 49             # tiles per core
NT = 128             # nodes per tile
NPC = TPC * NT       # 6272 nodes per core
NPAD = NCORES * NPC  # 50176
RW = 256             # Hfull row elems (bf16 -> 512 B)
ADR = 128            # adrep row elems (bf16 -> 256 B)
SPLIT = 32768        # max int16 gather index + 1
BASE_B = NPAD - SPLIT  # 17408; group-B gathers read Hfull[BASE_B:]

_CACHE = {}


def _to_bf16(a):
    """f32 -> bf16 (round-to-nearest-even)."""
    try:
        import ml_dtypes
        return np.asarray(a, dtype=np.float32).astype(ml_dtypes.bfloat16)
    except ImportError:
        x = np.ascontiguousarray(a, dtype=np.float32).view(np.uint32)
        rounded = (((x >> 16) + ((x >> 15) & 1)) & 0xFFFF).astype(np.uint16)
        return rounded


def _prep(edge_index):
    """Host-side edge preprocessing. Returns per-core index/metadata arrays."""
    src = np.asarray(edge_index[0], dtype=np.int64)
    dst = np.asarray(edge_index[1], dtype=np.int64)
    loop = np.arange(N, dtype=np.int64)
    src = np.concatenate([src, loop])
    dst = np.concatenate([dst, loop])

    tile_g = dst // NT                     # global tile id 0..391
    half = (src >= SPLIT).astype(np.int64)
    src_row = np.where(half == 1, src - BASE_B, src)
    order = np.lexsort((src, dst, half, tile_g))
    src_row = src_row[order]
    src, dst, tile_g, half = src[order], dst[order], tile_g[order], half[order]

    NTILES_G = NPAD // NT                  # 392
    cnt = np.zeros((NTILES_G, 2), dtype=np.int64)
    np.add.at(cnt, (tile_g, half), 1)
    starts = np.zeros((NTILES_G, 2), dtype=np.int64)
    starts.reshape(-1)[1:] = np.cumsum(cnt.reshape(-1))[:-1]

    # chunks per (slot, half): max over cores
    cores = np.arange(NCORES)
    cpt = np.zeros((TPC, 2), dtype=np.int64)
    for s in range(TPC):
        t_ids = cores * TPC + s
        for hf in range(2):
            cpt[s, hf] = max(1, int(np.ceil(cnt[t_ids, hf].max() / NT)))
    TOTC = int(cpt.sum())
    TOTIDX = TOTC * NT

    src_rel = np.zeros((NCORES, TOTC, NT), dtype=np.int64)
    ad_idx = np.zeros((NCORES, TOTC, NT), dtype=np.int64)
    dst_rel = np.full((NCORES, TOTC, NT), 999.0, dtype=np.float32)
    dloc_all = np.zeros((NCORES, TOTC, NT), dtype=np.int64)
    valid = np.zeros((NCORES, TOTC, NT), dtype=bool)

    for k in range(NCORES):
        coff = 0
        for s in range(TPC):
            t = k * TPC + s
            for hf in range(2):
                nch = int(cpt[s, hf])
                st, cn = starts[t, hf], int(cnt[t, hf])
                src_rel[k, coff:coff + nch].reshape(-1)[:cn] = src_row[st:st + cn]
                ad_idx[k, coff:coff + nch].reshape(-1)[:cn] = dst[st:st + cn] % NPC
                dloc_all[k, coff:coff + nch].reshape(-1)[:cn] = dst[st:st + cn] % NT
                valid[k, coff:coff + nch].reshape(-1)[:cn] = True
                coff += nch
        assert coff == TOTC

    assert src_rel.min() >= 0 and src_rel.max() < SPLIT

    # window offsets per chunk (uniform across cores): 64-wide at offsets
    # {0, 64} when the cross-core dst span fits, else full 128.
    woff = np.zeros(TOTC, dtype=np.int64)
    wlen = np.full(TOTC, 128, dtype=np.int64)
    for c in range(TOTC):
        v = valid[:, c, :]
        if v.any():
            dl = dloc_all[:, c, :][v]
            lo, hi = int(dl.min()), int(dl.max())
            wo = 0 if lo < 64 else 64
            if hi < wo + 64:
                woff[c] = wo
                wlen[c] = 64

    for k in range(NCORES):
        dr = dloc_all[k] - woff[:, None]
        dst_rel[k][valid[k]] = dr[valid[k]].astype(np.float32)

    # wrapped int16 index layout: [128, TOTIDX//16]; idx i of a call at
    # partition i%16 (replicated x8), col i//16. Calls slice columns.
    def wrap(a):  # a: [NCORES, TOTC, NT] -> [NCORES, 128, TOTIDX//16]
        fl = a.reshape(NCORES, TOTIDX)
        w = fl.reshape(NCORES, TOTIDX // 16, 16).transpose(0, 2, 1)
        return np.tile(w, (1, 8, 1)).astype(np.int16)

    src_w = wrap(src_rel)
    ad_w = wrap(ad_idx)
    # dst_rel for SBUF [128, TOTC]: partition=edge pos, col=chunk
    dst_col = dst_rel.transpose(0, 2, 1).copy()  # [NCORES, 128, TOTC]

    meta = {"cpt": cpt, "woff": woff, "wlen": wlen, "TOTC": TOTC,
            "TOTIDX": TOTIDX}
    return src_w, ad_w, dst_col, meta


def _build(meta):
    import concourse.bass as bass
    import concourse.bacc as bacc
    import concourse.mybir as mybir
    import concourse.tile as tile

    f32 = mybir.dt.float32
    f32r = mybir.dt.float32r
    bf16 = mybir.dt.bfloat16
    i16 = mybir.dt.int16
    i32 = mybir.dt.int32
    AF = mybir.ActivationFunctionType
    OP = mybir.AluOpType

    cpt, woff, wlen = meta["cpt"], meta["woff"], meta["wlen"]
    TOTC, TOTIDX = meta["TOTC"], meta["TOTIDX"]

    nc = bacc.Bacc("TRN2", target_bir_lowering=False, debug=False)

    # ---- I/O ----
    xT_in = nc.dram_tensor("xT_in", [NFP, NPC], bf16, kind="ExternalInput")
    idx_src = nc.dram_tensor("idx_src", [128, TOTIDX // 16], i16, kind="ExternalInput")
    idx_ad = nc.dram_tensor("idx_ad", [128, TOTIDX // 16], i16, kind="ExternalInput")
    dst_col = nc.dram_tensor("dst_col", [128, TOTC], f32, kind="ExternalInput")
    fb_p = nc.dram_tensor("fb_p", [NFP, NMEL], bf16, kind="ExternalInput")
    Wg_d = nc.dram_tensor("Wg", [NMEL, HC], f32, kind="ExternalInput")
    Wg_bf_d = nc.dram_tensor("Wg_bf", [NMEL, HC], bf16, kind="ExternalInput")
    attb_s = nc.dram_tensor("attb_s", [HC, 4], f32, kind="ExternalInput")
    attb_d = nc.dram_tensor("attb_d", [HC, 4], f32, kind="ExternalInput")
    bias_bc = nc.dram_tensor("bias_bc", [128, HC], f32, kind="ExternalInput")
    W1_d = nc.dram_tensor("W1", [HC, 256], bf16, kind="ExternalInput")
    b1_d = nc.dram_tensor("b1", [128, 2], f32, kind="ExternalInput")
    W2_d = nc.dram_tensor("W2", [256, HC], bf16, kind="ExternalInput")
    b2_d = nc.dram_tensor("b2", [128, 1], f32, kind="ExternalInput")
    W3_d = nc.dram_tensor("W3", [HC, 10], bf16, kind="ExternalInput")
    b3_d = nc.dram_tensor("b3", [128, 1], f32, kind="ExternalInput")
    eye_d = nc.dram_tensor("eye", [128, 128], f32, kind="ExternalInput")
    iota_d = nc.dram_tensor("iota", [128, 128], bf16, kind="ExternalInput")
    ones_d = nc.dram_tensor("ones", [128, 16], bf16, kind="ExternalInput")
    outT = nc.dram_tensor("outT", [10, NPC], f32, kind="ExternalOutput")

    core_ids = list(range(NCORES))

    with tile.TileContext(nc) as tc:
        with (
            tc.tile_pool(name="dram", bufs=1, space="DRAM") as dpool,
            tc.tile_pool(name="const", bufs=1) as cpool,
        ):
            Hext_loc = dpool.tile([NPC, RW], bf16)
            Hfull = dpool.tile([NPAD, RW], bf16, addr_space="Shared")
            adrep = dpool.tile([NPC, ADR], bf16)

            # ---- constants to SBUF ----
            fb_t = cpool.tile([128, 5, NMEL], bf16)
            nc.sync.dma_start(fb_t[:], fb_p.rearrange("(b p) m -> p b m", p=128))
            Wg_t = cpool.tile([128, HC], f32)
            nc.sync.dma_start(Wg_t[:], Wg_d[:])
            Wg_bf = cpool.tile([128, HC], bf16)
            nc.sync.dma_start(Wg_bf[:], Wg_bf_d[:])
            atts_t = cpool.tile([128, 4], f32)
            nc.sync.dma_start(atts_t[:], attb_s[:])
            attd_t = cpool.tile([128, 4], f32)
            nc.sync.dma_start(attd_t[:], attb_d[:])
            bias_t = cpool.tile([128, HC], f32)
            nc.sync.dma_start(bias_t[:], bias_bc[:])
            W1_t = cpool.tile([128, 256], bf16)
            nc.sync.dma_start(W1_t[:], W1_d[:])
            b1_t = cpool.tile([128, 2], f32)
            nc.sync.dma_start(b1_t[:], b1_d[:])
            W2_t = cpool.tile([128, 2, HC], bf16)
            nc.sync.dma_start(W2_t[:], W2_d.rearrange("(b p) m -> p b m", p=128))
            b2_t = cpool.tile([128, 1], f32)
            nc.sync.dma_start(b2_t[:], b2_d[:])
            W3_t = cpool.tile([128, 10], bf16)
            nc.sync.dma_start(W3_t[:], W3_d[:])
            b3_t = cpool.tile([128, 1], f32)
            nc.sync.dma_start(b3_t[:], b3_d[:])
            eye_t = cpool.tile([128, 128], f32)
            nc.sync.dma_start(eye_t[:], eye_d[:])
            iota_t = cpool.tile([128, 128], bf16)
            nc.sync.dma_start(iota_t[:], iota_d[:])
            ones_t = cpool.tile([128, 16], bf16)
            nc.sync.dma_start(ones_t[:], ones_d[:])
            isrc_t = cpool.tile([128, TOTIDX // 16], i16)
            nc.sync.dma_start(isrc_t[:], idx_src[:])
            iad_t = cpool.tile([128, TOTIDX // 16], i16)
            nc.sync.dma_start(iad_t[:], idx_ad[:])
            dcol_t = cpool.tile([128, TOTC], f32)
            nc.sync.dma_start(dcol_t[:], dst_col[:])

            # WgT, Wgatt_s/d
            WgT_t = cpool.tile([128, 128], f32)
            Wgatt_t = cpool.tile([128, 8], f32)
            Wgatt_bf = cpool.tile([128, 8], bf16)
            with tc.tile_pool(name="cpsum", bufs=1, space="PSUM") as cpsum:
                WgT_ps = cpsum.tile([128, 128], f32)
                nc.tensor.transpose(WgT_ps[:], Wg_t[:], eye_t[:])
                nc.vector.tensor_copy(WgT_t[:], WgT_ps[:])
                Wgatt_ps = cpsum.tile([128, 8], f32)
                nc.tensor.matmul(Wgatt_ps[:, 0:4], WgT_t[:], atts_t[:])
                nc.tensor.matmul(Wgatt_ps[:, 4:8], WgT_t[:], attd_t[:])
                nc.vector.tensor_copy(Wgatt_t[:], Wgatt_ps[:])
                nc.vector.tensor_copy(Wgatt_bf[:], Wgatt_ps[:])

            # ================= stage A =================
            with (
                tc.tile_pool(name="sa_sb", bufs=4) as sa,
                tc.tile_pool(name="sa_ps", bufs=3, space="PSUM") as saps,
                tc.tile_pool(name="sa_ps1", bufs=4, space="PSUM") as saps1,
            ):
                for g0 in range(0, TPC, 4):
                    gsz = min(4, TPC - g0)
                    gn = gsz * NT
                    xT = sa.tile([128, 5, 4 * NT], bf16, tag="xT")
                    nc.sync.dma_start(
                        xT[:, :, 0:gn],
                        xT_in.rearrange("(b p) n -> p b n", p=128)[
                            :, :, g0 * NT:g0 * NT + gn])
                    h1T_ps = saps.tile([128, 4 * NT], f32, tag="h1T")
                    for b in range(5):
                        nc.tensor.matmul(
                            h1T_ps[:, 0:gn],
                            fb_t[:, b, :],
                            xT[:, b, 0:gn],
                            start=(b == 0), stop=(b == 4))
                    h1T = sa.tile([128, 4 * NT], bf16, tag="h1Ts")
                    nc.scalar.activation(h1T[:, 0:gn], h1T_ps[:, 0:gn], AF.Copy)
                    for u in range(gsz):
                        h_ps = saps1.tile([128, HC + 8], f32, tag="hps")
                        lhs = h1T[:, u * NT:(u + 1) * NT]
                        nc.tensor.matmul(h_ps[:, 0:HC], lhs, Wg_bf[:])
                        nc.tensor.matmul(h_ps[:, HC:HC + 8], lhs, Wgatt_bf[:])
                        s = g0 + u
                        hrow = sa.tile([128, RW], bf16, tag="hrow")
                        nc.scalar.activation(hrow[:, 0:HC + 4],
                                             h_ps[:, 0:HC + 4], AF.Copy)
                        nc.sync.dma_start(
                            Hext_loc[s * NT:(s + 1) * NT, 0:HC + 4],
                            hrow[:, 0:HC + 4])
                        adr = sa.tile([128, ADR], bf16, tag="adr")
                        src_ap = bass.AP(
                            h_ps.tensor, h_ps.offset + (HC + 4),
                            [h_ps.ap[0], [0, ADR // 4], [1, 4]])
                        nc.scalar.activation(
                            adr[:].rearrange("p (a b) -> p a b",
                                             a=ADR // 4, b=4),
                            src_ap, AF.Copy)
                        nc.sync.dma_start(adrep[s * NT:(s + 1) * NT, :], adr[:])

                nc.gpsimd.collective_compute(
                    "AllGather", mybir.AluOpType.bypass,
                    ins=[Hext_loc[:]], outs=[Hfull[:]],
                    replica_groups=[core_ids])

            # ================= edge phase + MLP =================
            coffs = np.concatenate([[0], np.cumsum(cpt.sum(axis=1))]).astype(int)
            CPTA_MAX = int(cpt[:, 0].max())
            CPTB_MAX = int(cpt[:, 1].max())
            TOT_MAX = int((cpt[:, 0] + cpt[:, 1]).max())

            with (
                tc.tile_pool(name="eg_g", bufs=3) as egg,
                tc.tile_pool(name="eg_sb", bufs=2) as egs,
                tc.tile_pool(name="eg_acc", bufs=2, space="PSUM") as egacc,
                tc.tile_pool(name="eg_tp", bufs=2, space="PSUM") as egtp,
                tc.tile_pool(name="mlp_sb", bufs=2) as msb,
                tc.tile_pool(name="mlp_ps", bufs=1, space="PSUM") as mps,
            ):
                actT4 = None
                gsz = 4
                for s in range(TPC):
                    cA, cB = int(cpt[s, 0]), int(cpt[s, 1])
                    tot = cA + cB
                    coff = int(coffs[s])

                    acc = egacc.tile([128, 132], f32, tag="acc")
                    nc.vector.memset(acc[:], 0.0)

                    ad = egg.tile([128, TOT_MAX, ADR], bf16, tag="ad")
                    nc.gpsimd.dma_gather(
                        ad[:, 0:tot, :], adrep[:],
                        iad_t[:, coff * 8:(coff + tot) * 8],
                        num_idxs=tot * NT, num_idxs_reg=tot * NT,
                        elem_size=ADR, single_packet=False)

                    halves = []
                    gA = egg.tile([128, CPTA_MAX, RW], bf16, tag="gA")
                    nc.gpsimd.dma_gather(
                        gA[:, 0:cA, :], Hfull[:],
                        isrc_t[:, coff * 8:(coff + cA) * 8],
                        num_idxs=cA * NT, num_idxs_reg=cA * NT, elem_size=RW,
                        single_packet=False)
                    halves.append((gA, 0, cA))
                    gB = egg.tile([128, CPTB_MAX, RW], bf16, tag="gB")
                    nc.gpsimd.dma_gather(
                        gB[:, 0:cB, :], Hfull[BASE_B:NPAD, :],
                        isrc_t[:, (coff + cA) * 8:(coff + tot) * 8],
                        num_idxs=cB * NT, num_idxs_reg=cB * NT, elem_size=RW,
                        single_packet=False)
                    halves.append((gB, cA, cB))

                    ind = egs.tile([128, TOT_MAX, 128], bf16, tag="ind")
                    ind = egs.tile([128, TOT_MAX, 128], bf16, tag="ind")
                    for (gt, c0, nh) in halves:
                        if nh == 0:
                            continue
                        # t = a_s + a_d  -> ex = exp(prelu(t))
                        ex = egs.tile([128, TOT_MAX, 4], f32, tag="ex", bufs=2)
                        nc.vector.tensor_tensor(
                            ex[:, 0:nh, :], gt[:, 0:nh, 128:132],
                            ad[:, c0:c0 + nh, 0:4], OP.add)
                        nc.scalar.activation(
                            ex[:, 0:nh, :], ex[:, 0:nh, :], AF.Prelu,
                            alpha=NEG_ATT)
                        nc.scalar.activation(
                            ex[:, 0:nh, :], ex[:, 0:nh, :], AF.Exp)
                        exs = egs.tile([128, TOT_MAX, 4], bf16, tag="exs", bufs=2)
                        nc.vector.tensor_copy(exs[:, 0:nh, :], ex[:, 0:nh, :])
                        # msg *= ex (per head block)
                        g4 = bass.AP(
                            gt.tensor, gt.offset,
                            [gt.ap[0], [RW, nh], [32, 4], [1, 32]])
                        exb = bass.AP(
                            exs.tensor, exs.offset,
                            [exs.ap[0], [4, nh], [1, 4], [0, 32]])
                        nc.vector.tensor_tensor(g4, g4, exb, OP.mult)
                        # ex -> cols 128:132 (scalar engine)
                        nc.scalar.activation(
                            gt[:, 0:nh, 128:132], exs[:, 0:nh, :], AF.Copy)
                        # indicator (window-relative) per chunk + accumulate
                        for c in range(nh):
                            wo = int(woff[coff + c0 + c])
                            wl = int(wlen[coff + c0 + c])
                            nc.vector.tensor_scalar(
                                ind[:, c0 + c, 0:wl], iota_t[:, 0:wl],
                                dcol_t[:, coff + c0 + c:coff + c0 + c + 1],
                                None, OP.is_equal)
                            nc.tensor.matmul(
                                acc[wo:wo + wl, :],
                                ind[:, c0 + c, 0:wl], gt[:, c, 0:132],
                                start=False, stop=(c0 + c == tot - 1),
                                skip_group_check=True)

                    # normalize + bias + ELU (node-major)
                    dinv = egs.tile([128, 4], f32, tag="dinv")
                    nc.vector.reciprocal(dinv[:], acc[:, 128:132])
                    gat = egs.tile([128, 128], f32, tag="gat")
                    ga = bass.AP(gat.tensor, gat.offset,
                                 [gat.ap[0], [32, 4], [1, 32]])
                    aa = bass.AP(acc.tensor, acc.offset,
                                 [acc.ap[0], [32, 4], [1, 32]])
                    db = bass.AP(dinv.tensor, dinv.offset,
                                 [dinv.ap[0], [1, 4], [0, 32]])
                    nc.vector.tensor_tensor(ga, aa, db, OP.mult)
                    nc.vector.tensor_tensor(gat[:], gat[:], bias_t[:], OP.add)
                    # ELU = relu(x) - relu(1 - exp(x))
                    t1 = egs.tile([128, 128], f32, tag="t1")
                    nc.scalar.activation(t1[:], gat[:], AF.Exp)
                    nc.scalar.activation(t1[:], t1[:], AF.Relu, scale=-1.0, bias=1.0)
                    nc.scalar.activation(gat[:], gat[:], AF.Relu)
                    nc.vector.tensor_sub(gat[:], gat[:], t1[:])
                    # transpose -> actT4
                    sub = s % 4
                    if sub == 0:
                        gsz = min(4, TPC - s)
                        actT4 = msb.tile([128, 4 * NT], bf16, tag="actT4")
                    tp = egtp.tile([128, 128], f32, tag="tp2", bufs=1)
                    nc.tensor.transpose(tp[:], gat[:], eye_t[:])
                    nc.scalar.activation(actT4[:, sub * NT:(sub + 1) * NT],
                                         tp[:], AF.Copy)

                    if sub == gsz - 1:
                        g0 = s - sub
                        gn = gsz * NT
                        a1 = msb.tile([128, 2, 512], bf16, tag="a1")
                        for j in range(2):
                            o1 = mps.tile([128, 512], f32, tag="o1")
                            nc.tensor.matmul(
                                o1[:, 0:gn],
                                W1_t[:, j * 128:(j + 1) * 128],
                                actT4[:, 0:gn])
                            nc.scalar.activation(
                                a1[:, j, 0:gn], o1[:, 0:gn], AF.Prelu,
                                alpha=NEG_MLP, bias=b1_t[:, j:j + 1])
                        o2 = mps.tile([128, 512], f32, tag="o2")
                        for j in range(2):
                            nc.tensor.matmul(
                                o2[:, 0:gn], W2_t[:, j, :],
                                a1[:, j, 0:gn],
                                start=(j == 0), stop=(j == 1))
                        a2 = msb.tile([128, 512], bf16, tag="a2")
                        nc.scalar.activation(
                            a2[:, 0:gn], o2[:, 0:gn], AF.Prelu,
                            alpha=NEG_MLP, bias=b2_t[:])
                        o3 = mps.tile([16, 512], f32, tag="sm", name="o3_t")
                        nc.tensor.matmul(o3[0:10, 0:gn], W3_t[:],
                                         a2[:, 0:gn])
                        z = msb.tile([16, 512], bf16, tag="z")
                        nc.scalar.activation(
                            z[0:10, 0:gn], o3[0:10, 0:gn], AF.Prelu,
                            alpha=NEG_MLP, bias=b3_t[0:10, :])
                        nc.scalar.activation(z[0:10, 0:gn], z[0:10, 0:gn], AF.Exp)
                        ssum = mps.tile([16, 512], f32, tag="sm", name="ssum_t")[0:1, :]
                        nc.tensor.matmul(
                            ssum[:, 0:gn], ones_t[0:10, 0:1],
                            z[0:10, 0:gn])
                        sinv = msb.tile([1, 512], bf16, tag="sinv")
                        with nc.allow_low_precision(reason="softmax denom bf16"):
                            nc.vector.reciprocal(sinv[:, 0:gn], ssum[:, 0:gn])
                        sx = mps.tile([16, 512], f32, tag="sm", name="sx_t")
                        nc.tensor.matmul(
                            sx[0:10, 0:gn], ones_t[0:1, 0:10],
                            sinv[:, 0:gn])
                        res = msb.tile([16, 512], f32, tag="res")
                        nc.vector.tensor_mul(
                            res[0:10, 0:gn], z[0:10, 0:gn], sx[0:10, 0:gn])
                        nc.sync.dma_start(
                            outT[:, g0 * NT:g0 * NT + gn], res[0:10, 0:gn])

    nc.compile()
    return nc


def _inputs_per_core(inputs, src_w, ad_w, dst_col, meta):
    x = np.asarray(inputs["x"], dtype=np.float32)
    fb = np.asarray(inputs["fb"], dtype=np.float32)
    Wg = np.asarray(inputs["Wg"], dtype=np.float32)
    bias_g = np.asarray(inputs["bias_g"], dtype=np.float32)
    att_src = np.asarray(inputs["att_src"], dtype=np.float32)
    att_dst = np.asarray(inputs["att_dst"], dtype=np.float32)
    W1 = np.asarray(inputs["W1"], dtype=np.float32)
    b1 = np.asarray(inputs["b1"], dtype=np.float32)
    W2 = np.asarray(inputs["W2"], dtype=np.float32)
    b2 = np.asarray(inputs["b2"], dtype=np.float32)
    W3 = np.asarray(inputs["W3"], dtype=np.float32)
    b3 = np.asarray(inputs["b3"], dtype=np.float32)

    x_pad = np.zeros((NPAD, NFP), dtype=np.float32)
    x_pad[:N, :NF] = x
    fb_pad = np.zeros((NFP, NMEL), dtype=np.float32)
    fb_pad[:NF] = fb

    att_blk_s = np.zeros((HC, 4), dtype=np.float32)
    att_blk_d = np.zeros((HC, 4), dtype=np.float32)
    for h in range(H):
        att_blk_s[h * C:(h + 1) * C, h] = att_src[h]
        att_blk_d[h * C:(h + 1) * C, h] = att_dst[h]

    b1p = np.zeros((128, 2), dtype=np.float32)
    b1p[:, 0] = b1[:128]
    b1p[:, 1] = b1[128:]
    b2p = b2.reshape(128, 1).astype(np.float32)
    b3p = np.zeros((128, 1), dtype=np.float32)
    b3p[:10, 0] = b3

    iota_f32 = np.tile(np.arange(128, dtype=np.float32)[None, :], (128, 1))
    common = {
        "fb_p": _to_bf16(fb_pad), "Wg": Wg, "Wg_bf": _to_bf16(Wg),
        "attb_s": att_blk_s, "attb_d": att_blk_d,
        "bias_bc": np.tile(bias_g[None, :], (128, 1)).astype(np.float32),
        "W1": _to_bf16(W1), "b1": b1p, "W2": _to_bf16(W2), "b2": b2p,
        "W3": _to_bf16(W3), "b3": b3p,
        "eye": np.eye(128, dtype=np.float32),
        "eyebf": _to_bf16(np.eye(128, dtype=np.float32)),
        "iota": _to_bf16(iota_f32),
        "ones": _to_bf16(np.ones((128, 16), dtype=np.float32)),
    }

    maps = []
    for k in range(NCORES):
        m = dict(common)
        m["xT_in"] = _to_bf16(
            np.ascontiguousarray(x_pad[k * NPC:(k + 1) * NPC].T))
        m["idx_src"] = src_w[k]
        m["idx_ad"] = ad_w[k]
        m["dst_col"] = dst_col[k].astype(np.float32)
        maps.append(m)
    return maps


def kernel(**inputs):
    from concourse.bass_utils import run_bass_kernel_spmd

    src_w, ad_w, dst_col, meta = _prep(inputs["edge_index"])
    key = ("nc", meta["TOTC"], tuple(meta["cpt"].reshape(-1)),
           tuple(meta["woff"]))
    if key not in _CACHE:
        _CACHE.clear()
        _CACHE[key] = _build(meta)
    nc = _CACHE[key]
    maps = _inputs_per_core(inputs, src_w, ad_w, dst_col, meta)
    res = run_bass_kernel_spmd(nc, maps, core_ids=list(range(NCORES)))
    out = np.zeros((NPAD, 10), dtype=np.float32)
    for k in range(NCORES):
        out[k * NPC:(k + 1) * NPC] = res.results[k]["outT"].T
    return out[:N]


# revision 34
# speedup vs baseline: 1.0094x; 1.0094x over previous
"""GAT (gnn_message_passing) Trainium2 Bass kernel — 8-core SPMD.

Contract: kernel(**inputs) -> np.ndarray with FULL inputs / FULL output.
Self-contained: hardcodes shapes; only imports the container's concourse stack.

Design:
- Stage A: per-core h = (x @ fb) @ Wg plus attention dots, written directly
  into one Shared-DRAM node table via indexed (indirect) DMA writes.
- A tiny barrier AllGather replaces the two big halo AllGathers.
- Edge phase: per 128-node destination tile, gather source rows (bf16,
  512B/row) + dst attention rows, compute edge softmax numerators on
  DVE/Act, accumulate per-node sums with indicator matmuls in PSUM,
  normalize, then fused 3-layer MLP + softmax.
"""
import sys

for _p in ("/opt/trn_rl_repo", "/root/.axon_site/_ro/trn_rl_repo"):
    if _p not in sys.path:
        sys.path.append(_p)

import numpy as np

# ---------------- problem constants (hardcoded per contract) ----------------
N = 50000
NF = 513
NFP = 640            # padded feature dim (5 * 128)
NMEL = 128
H, C = 4, 32
HC = H * C           # 128
E = 800000
NEG_ATT = 0.2
NEG_MLP = 0.01

NCORES = 8
TPC = 49             # tiles per core
NT = 128             # nodes per tile
NPC = TPC * NT       # 6272 nodes per core
NPAD = NCORES * NPC  # 50176
RW = 256             # Hfull row elems (bf16 -> 512 B)
ADR = 128            # adrep row elems (bf16 -> 256 B)
SPLIT = 32768        # max int16 gather index + 1
BASE_B = NPAD - SPLIT  # 17408; group-B gathers read Hfull[BASE_B:]

_CACHE = {}


def _to_bf16(a):
    """f32 -> bf16 (round-to-nearest-even)."""
    try:
        import ml_dtypes
        return np.asarray(a, dtype=np.float32).astype(ml_dtypes.bfloat16)
    except ImportError:
        x = np.ascontiguousarray(a, dtype=np.float32).view(np.uint32)
        rounded = (((x >> 16) + ((x >> 15) & 1)) & 0xFFFF).astype(np.uint16)
        return rounded


def _prep(edge_index):
    """Host-side edge preprocessing. Returns per-core index/metadata arrays."""
    src = np.asarray(edge_index[0], dtype=np.int64)
    dst = np.asarray(edge_index[1], dtype=np.int64)
    loop = np.arange(N, dtype=np.int64)
    src = np.concatenate([src, loop])
    dst = np.concatenate([dst, loop])

    tile_g = dst // NT                     # global tile id 0..391
    half = (src >= SPLIT).astype(np.int64)
    src_row = np.where(half == 1, src - BASE_B, src)
    order = np.lexsort((src, dst, half, tile_g))
    src_row = src_row[order]
    src, dst, tile_g, half = src[order], dst[order], tile_g[order], half[order]

    NTILES_G = NPAD // NT                  # 392
    cnt = np.zeros((NTILES_G, 2), dtype=np.int64)
    np.add.at(cnt, (tile_g, half), 1)
    starts = np.zeros((NTILES_G, 2), dtype=np.int64)
    starts.reshape(-1)[1:] = np.cumsum(cnt.reshape(-1))[:-1]

    # chunks per (slot, half): max over cores
    cores = np.arange(NCORES)
    cpt = np.zeros((TPC, 2), dtype=np.int64)
    for s in range(TPC):
        t_ids = cores * TPC + s
        for hf in range(2):
            cpt[s, hf] = max(1, int(np.ceil(cnt[t_ids, hf].max() / NT)))
    TOTC = int(cpt.sum())
    TOTIDX = TOTC * NT

    src_rel = np.zeros((NCORES, TOTC, NT), dtype=np.int64)
    ad_idx = np.zeros((NCORES, TOTC, NT), dtype=np.int64)
    dst_rel = np.full((NCORES, TOTC, NT), 999.0, dtype=np.float32)
    dloc_all = np.zeros((NCORES, TOTC, NT), dtype=np.int64)
    valid = np.zeros((NCORES, TOTC, NT), dtype=bool)

    for k in range(NCORES):
        coff = 0
        for s in range(TPC):
            t = k * TPC + s
            for hf in range(2):
                nch = int(cpt[s, hf])
                st, cn = starts[t, hf], int(cnt[t, hf])
                src_rel[k, coff:coff + nch].reshape(-1)[:cn] = src_row[st:st + cn]
                ad_idx[k, coff:coff + nch].reshape(-1)[:cn] = dst[st:st + cn] % NPC
                dloc_all[k, coff:coff + nch].reshape(-1)[:cn] = dst[st:st + cn] % NT
                valid[k, coff:coff + nch].reshape(-1)[:cn] = True
                coff += nch
        assert coff == TOTC

    assert src_rel.min() >= 0 and src_rel.max() < SPLIT

    # window offsets per chunk (uniform across cores): 64-wide at offsets
    # {0, 64} when the cross-core dst span fits, else full 128.
    woff = np.zeros(TOTC, dtype=np.int64)
    wlen = np.full(TOTC, 128, dtype=np.int64)
    for c in range(TOTC):
        v = valid[:, c, :]
        if v.any():
            dl = dloc_all[:, c, :][v]
            lo, hi = int(dl.min()), int(dl.max())
            wo = 0 if lo < 64 else 64
            if hi < wo + 64:
                woff[c] = wo
                wlen[c] = 64

    for k in range(NCORES):
        dr = dloc_all[k] - woff[:, None]
        dst_rel[k][valid[k]] = dr[valid[k]].astype(np.float32)

    # wrapped int16 index layout: [128, TOTIDX//16]; idx i of a call at
    # partition i%16 (replicated x8), col i//16. Calls slice columns.
    def wrap(a):  # a: [NCORES, TOTC, NT] -> [NCORES, 128, TOTIDX//16]
        fl = a.reshape(NCORES, TOTIDX)
        w = fl.reshape(NCORES, TOTIDX // 16, 16).transpose(0, 2, 1)
        return np.tile(w, (1, 8, 1)).astype(np.int16)

    src_w = wrap(src_rel)
    ad_w = wrap(ad_idx)
    # dst_rel for SBUF [128, TOTC]: partition=edge pos, col=chunk
    dst_col = dst_rel.transpose(0, 2, 1).copy()  # [NCORES, 128, TOTC]

    meta = {"cpt": cpt, "woff": woff, "wlen": wlen, "TOTC": TOTC,
            "TOTIDX": TOTIDX}
    return src_w, ad_w, dst_col, meta


def _build(meta):
    import concourse.bass as bass
    import concourse.bacc as bacc
    import concourse.mybir as mybir
    import concourse.tile as tile

    f32 = mybir.dt.float32
    f32r = mybir.dt.float32r
    bf16 = mybir.dt.bfloat16
    i16 = mybir.dt.int16
    i32 = mybir.dt.int32
    AF = mybir.ActivationFunctionType
    OP = mybir.AluOpType

    cpt, woff, wlen = meta["cpt"], meta["woff"], meta["wlen"]
    TOTC, TOTIDX = meta["TOTC"], meta["TOTIDX"]

    nc = bacc.Bacc("TRN2", target_bir_lowering=False, debug=False)

    # ---- I/O ----
    xT_in = nc.dram_tensor("xT_in", [NFP, NPC], bf16, kind="ExternalInput")
    idx_src = nc.dram_tensor("idx_src", [128, TOTIDX // 16], i16, kind="ExternalInput")
    idx_ad = nc.dram_tensor("idx_ad", [128, TOTIDX // 16], i16, kind="ExternalInput")
    dst_col = nc.dram_tensor("dst_col", [128, TOTC], f32, kind="ExternalInput")
    fb_p = nc.dram_tensor("fb_p", [NFP, NMEL], bf16, kind="ExternalInput")
    Wg_d = nc.dram_tensor("Wg", [NMEL, HC], f32, kind="ExternalInput")
    Wg_bf_d = nc.dram_tensor("Wg_bf", [NMEL, HC], bf16, kind="ExternalInput")
    attb_s = nc.dram_tensor("attb_s", [HC, 4], f32, kind="ExternalInput")
    attb_d = nc.dram_tensor("attb_d", [HC, 4], f32, kind="ExternalInput")
    bias_bc = nc.dram_tensor("bias_bc", [128, HC], f32, kind="ExternalInput")
    W1_d = nc.dram_tensor("W1", [HC, 256], bf16, kind="ExternalInput")
    b1_d = nc.dram_tensor("b1", [128, 2], f32, kind="ExternalInput")
    W2_d = nc.dram_tensor("W2", [256, HC], bf16, kind="ExternalInput")
    b2_d = nc.dram_tensor("b2", [128, 1], f32, kind="ExternalInput")
    W3_d = nc.dram_tensor("W3", [HC, 10], bf16, kind="ExternalInput")
    b3_d = nc.dram_tensor("b3", [128, 1], f32, kind="ExternalInput")
    eye_d = nc.dram_tensor("eye", [128, 128], f32, kind="ExternalInput")
    iota_d = nc.dram_tensor("iota", [128, 128], bf16, kind="ExternalInput")
    ones_d = nc.dram_tensor("ones", [128, 16], bf16, kind="ExternalInput")
    outT = nc.dram_tensor("outT", [10, NPC], f32, kind="ExternalOutput")

    core_ids = list(range(NCORES))

    with tile.TileContext(nc) as tc:
        with (
            tc.tile_pool(name="dram", bufs=1, space="DRAM") as dpool,
            tc.tile_pool(name="const", bufs=1) as cpool,
        ):
            Hext_loc = dpool.tile([NPC, RW], bf16)
            Hfull = dpool.tile([NPAD, RW], bf16, addr_space="Shared")
            adrep = dpool.tile([NPC, ADR], bf16)

            # ---- constants to SBUF ----
            fb_t = cpool.tile([128, 5, NMEL], bf16)
            nc.sync.dma_start(fb_t[:], fb_p.rearrange("(b p) m -> p b m", p=128))
            Wg_t = cpool.tile([128, HC], f32)
            nc.sync.dma_start(Wg_t[:], Wg_d[:])
            Wg_bf = cpool.tile([128, HC], bf16)
            nc.sync.dma_start(Wg_bf[:], Wg_bf_d[:])
            atts_t = cpool.tile([128, 4], f32)
            nc.sync.dma_start(atts_t[:], attb_s[:])
            attd_t = cpool.tile([128, 4], f32)
            nc.sync.dma_start(attd_t[:], attb_d[:])
            bias_t = cpool.tile([128, HC], f32)
            nc.sync.dma_start(bias_t[:], bias_bc[:])
            W1_t = cpool.tile([128, 256], bf16)
            nc.sync.dma_start(W1_t[:], W1_d[:])
            b1_t = cpool.tile([128, 2], f32)
            nc.sync.dma_start(b1_t[:], b1_d[:])
            W2_t = cpool.tile([128, 2, HC], bf16)
            nc.sync.dma_start(W2_t[:], W2_d.rearrange("(b p) m -> p b m", p=128))
            b2_t = cpool.tile([128, 1], f32)
            nc.sync.dma_start(b2_t[:], b2_d[:])
            W3_t = cpool.tile([128, 10], bf16)
            nc.sync.dma_start(W3_t[:], W3_d[:])
            b3_t = cpool.tile([128, 1], f32)
            nc.sync.dma_start(b3_t[:], b3_d[:])
            eye_t = cpool.tile([128, 128], f32)
            nc.sync.dma_start(eye_t[:], eye_d[:])
            iota_t = cpool.tile([128, 128], bf16)
            nc.sync.dma_start(iota_t[:], iota_d[:])
            ones_t = cpool.tile([128, 16], bf16)
            nc.sync.dma_start(ones_t[:], ones_d[:])
            isrc_t = cpool.tile([128, TOTIDX // 16], i16)
            nc.sync.dma_start(isrc_t[:], idx_src[:])
            iad_t = cpool.tile([128, TOTIDX // 16], i16)
            nc.sync.dma_start(iad_t[:], idx_ad[:])
            dcol_t = cpool.tile([128, TOTC], f32)
            nc.sync.dma_start(dcol_t[:], dst_col[:])

            # WgT, Wgatt_s/d
            WgT_t = cpool.tile([128, 128], f32)
            Wgatt_t = cpool.tile([128, 8], f32)
            Wgatt_bf = cpool.tile([128, 8], bf16)
            with tc.tile_pool(name="cpsum", bufs=1, space="PSUM") as cpsum:
                WgT_ps = cpsum.tile([128, 128], f32)
                nc.tensor.transpose(WgT_ps[:], Wg_t[:], eye_t[:])
                nc.vector.tensor_copy(WgT_t[:], WgT_ps[:])
                Wgatt_ps = cpsum.tile([128, 8], f32)
                nc.tensor.matmul(Wgatt_ps[:, 0:4], WgT_t[:], atts_t[:])
                nc.tensor.matmul(Wgatt_ps[:, 4:8], WgT_t[:], attd_t[:])
                nc.vector.tensor_copy(Wgatt_t[:], Wgatt_ps[:])
                nc.vector.tensor_copy(Wgatt_bf[:], Wgatt_ps[:])

            # ================= stage A =================
            with (
                tc.tile_pool(name="sa_sb", bufs=4) as sa,
                tc.tile_pool(name="sa_ps", bufs=3, space="PSUM") as saps,
                tc.tile_pool(name="sa_ps1", bufs=4, space="PSUM") as saps1,
            ):
                for g0 in range(0, TPC, 4):
                    gsz = min(4, TPC - g0)
                    gn = gsz * NT
                    xT = sa.tile([128, 5, 4 * NT], bf16, tag="xT")
                    nc.sync.dma_start(
                        xT[:, :, 0:gn],
                        xT_in.rearrange("(b p) n -> p b n", p=128)[
                            :, :, g0 * NT:g0 * NT + gn])
                    h1T_ps = saps.tile([128, 4 * NT], f32, tag="h1T")
                    for b in range(5):
                        nc.tensor.matmul(
                            h1T_ps[:, 0:gn],
                            fb_t[:, b, :],
                            xT[:, b, 0:gn],
                            start=(b == 0), stop=(b == 4))
                    h1T = sa.tile([128, 4 * NT], bf16, tag="h1Ts")
                    nc.scalar.activation(h1T[:, 0:gn], h1T_ps[:, 0:gn], AF.Copy)
                    for u in range(gsz):
                        h_ps = saps1.tile([128, HC + 8], f32, tag="hps")
                        lhs = h1T[:, u * NT:(u + 1) * NT]
                        nc.tensor.matmul(h_ps[:, 0:HC], lhs, Wg_bf[:])
                        nc.tensor.matmul(h_ps[:, HC:HC + 8], lhs, Wgatt_bf[:])
                        s = g0 + u
                        hrow = sa.tile([128, RW], bf16, tag="hrow")
                        nc.scalar.activation(hrow[:, 0:HC + 4],
                                             h_ps[:, 0:HC + 4], AF.Copy)
                        nc.sync.dma_start(
                            Hext_loc[s * NT:(s + 1) * NT, 0:HC + 4],
                            hrow[:, 0:HC + 4])
                        adr = sa.tile([128, ADR], bf16, tag="adr")
                        src_ap = bass.AP(
                            h_ps.tensor, h_ps.offset + (HC + 4),
                            [h_ps.ap[0], [0, ADR // 4], [1, 4]])
                        nc.scalar.activation(
                            adr[:].rearrange("p (a b) -> p a b",
                                             a=ADR // 4, b=4),
                            src_ap, AF.Copy)
                        nc.sync.dma_start(adrep[s * NT:(s + 1) * NT, :], adr[:])

                nc.gpsimd.collective_compute(
                    "AllGather", mybir.AluOpType.bypass,
                    ins=[Hext_loc[:]], outs=[Hfull[:]],
                    replica_groups=[core_ids])

            # ================= edge phase + MLP =================
            coffs = np.concatenate([[0], np.cumsum(cpt.sum(axis=1))]).astype(int)
            CPTA_MAX = int(cpt[:, 0].max())
            CPTB_MAX = int(cpt[:, 1].max())
            TOT_MAX = int((cpt[:, 0] + cpt[:, 1]).max())

            with (
                tc.tile_pool(name="eg_g", bufs=3) as egg,
                tc.tile_pool(name="eg_sb", bufs=2) as egs,
                tc.tile_pool(name="eg_acc", bufs=2, space="PSUM") as egacc,
                tc.tile_pool(name="eg_tp", bufs=2, space="PSUM") as egtp,
                tc.tile_pool(name="mlp_sb", bufs=2) as msb,
                tc.tile_pool(name="mlp_ps", bufs=1, space="PSUM") as mps,
            ):
                actT4 = None
                gsz = 4
                for s in range(TPC):
                    cA, cB = int(cpt[s, 0]), int(cpt[s, 1])
                    tot = cA + cB
                    coff = int(coffs[s])

                    acc = egacc.tile([128, 132], f32, tag="acc")
                    nc.vector.memset(acc[:], 0.0)

                    ad = egg.tile([128, TOT_MAX, ADR], bf16, tag="ad")
                    nc.gpsimd.dma_gather(
                        ad[:, 0:tot, :], adrep[:],
                        iad_t[:, coff * 8:(coff + tot) * 8],
                        num_idxs=tot * NT, num_idxs_reg=tot * NT,
                        elem_size=ADR, single_packet=False)

                    halves = []
                    gA = egg.tile([128, CPTA_MAX, RW], bf16, tag="gA")
                    nc.gpsimd.dma_gather(
                        gA[:, 0:cA, :], Hfull[:],
                        isrc_t[:, coff * 8:(coff + cA) * 8],
                        num_idxs=cA * NT, num_idxs_reg=cA * NT, elem_size=RW,
                        single_packet=False)
                    halves.append((gA, 0, cA))
                    gB = egg.tile([128, CPTB_MAX, RW], bf16, tag="gB")
                    nc.gpsimd.dma_gather(
                        gB[:, 0:cB, :], Hfull[BASE_B:NPAD, :],
                        isrc_t[:, (coff + cA) * 8:(coff + tot) * 8],
                        num_idxs=cB * NT, num_idxs_reg=cB * NT, elem_size=RW,
                        single_packet=False)
                    halves.append((gB, cA, cB))

                    ind = egs.tile([128, TOT_MAX, 128], bf16, tag="ind")
                    ind = egs.tile([128, TOT_MAX, 128], bf16, tag="ind")
                    for (gt, c0, nh) in halves:
                        if nh == 0:
                            continue
                        # t = a_s + a_d  -> ex = exp(prelu(t))
                        ex = egs.tile([128, TOT_MAX, 4], f32, tag="ex", bufs=2)
                        nc.vector.tensor_tensor(
                            ex[:, 0:nh, :], gt[:, 0:nh, 128:132],
                            ad[:, c0:c0 + nh, 0:4], OP.add)
                        nc.scalar.activation(
                            ex[:, 0:nh, :], ex[:, 0:nh, :], AF.Prelu,
                            alpha=NEG_ATT)
                        nc.scalar.activation(
                            ex[:, 0:nh, :], ex[:, 0:nh, :], AF.Exp)
                        exs = egs.tile([128, TOT_MAX, 4], bf16, tag="exs", bufs=2)
                        nc.vector.tensor_copy(exs[:, 0:nh, :], ex[:, 0:nh, :])
                        # msg *= ex (per head block)
                        g4 = bass.AP(
                            gt.tensor, gt.offset,
                            [gt.ap[0], [RW, nh], [32, 4], [1, 32]])
                        exb = bass.AP(
                            exs.tensor, exs.offset,
                            [exs.ap[0], [4, nh], [1, 4], [0, 32]])
                        nc.vector.tensor_tensor(g4, g4, exb, OP.mult)
                        # ex -> cols 128:132 (scalar engine)
                        nc.scalar.activation(
                            gt[:, 0:nh, 128:132], exs[:, 0:nh, :], AF.Copy)
                        # indicator (window-relative) per chunk + accumulate
                        for c in range(nh):
                            wo = int(woff[coff + c0 + c])
                            wl = int(wlen[coff + c0 + c])
                            nc.vector.tensor_scalar(
                                ind[:, c0 + c, 0:wl], iota_t[:, 0:wl],
                                dcol_t[:, coff + c0 + c:coff + c0 + c + 1],
                                None, OP.is_equal)
                            nc.tensor.matmul(
                                acc[wo:wo + wl, :],
                                ind[:, c0 + c, 0:wl], gt[:, c, 0:132],
                                start=False, stop=(c0 + c == tot - 1),
                                skip_group_check=True)

                    # normalize + bias + ELU (node-major)
                    dinv = egs.tile([128, 4], f32, tag="dinv")
                    nc.vector.reciprocal(dinv[:], acc[:, 128:132])
                    gat = egs.tile([128, 128], f32, tag="gat")
                    for h in range(4):
                        nc.scalar.activation(
                            gat[:, h * 32:(h + 1) * 32],
                            acc[:, h * 32:(h + 1) * 32], AF.Copy,
                            scale=dinv[:, h:h + 1])
                    nc.vector.tensor_tensor(gat[:], gat[:], bias_t[:], OP.add)
                    # ELU = relu(x) - relu(1 - exp(x))
                    t1 = egs.tile([128, 128], f32, tag="t1")
                    nc.scalar.activation(t1[:], gat[:], AF.Exp)
                    nc.scalar.activation(t1[:], t1[:], AF.Relu, scale=-1.0, bias=1.0)
                    nc.scalar.activation(gat[:], gat[:], AF.Relu)
                    nc.vector.tensor_sub(gat[:], gat[:], t1[:])
                    # transpose -> actT4
                    sub = s % 4
                    if sub == 0:
                        gsz = min(4, TPC - s)
                        actT4 = msb.tile([128, 4 * NT], bf16, tag="actT4")
                    tp = egtp.tile([128, 128], f32, tag="tp2", bufs=1)
                    nc.tensor.transpose(tp[:], gat[:], eye_t[:])
                    nc.scalar.activation(actT4[:, sub * NT:(sub + 1) * NT],
                                         tp[:], AF.Copy)

                    if sub == gsz - 1:
                        g0 = s - sub
                        gn = gsz * NT
                        a1 = msb.tile([128, 2, 512], bf16, tag="a1")
                        for j in range(2):
                            o1 = mps.tile([128, 512], f32, tag="o1")
                            nc.tensor.matmul(
                                o1[:, 0:gn],
                                W1_t[:, j * 128:(j + 1) * 128],
                                actT4[:, 0:gn])
                            nc.scalar.activation(
                                a1[:, j, 0:gn], o1[:, 0:gn], AF.Prelu,
                                alpha=NEG_MLP, bias=b1_t[:, j:j + 1])
                        o2 = mps.tile([128, 512], f32, tag="o2")
                        for j in range(2):
                            nc.tensor.matmul(
                                o2[:, 0:gn], W2_t[:, j, :],
                                a1[:, j, 0:gn],
                                start=(j == 0), stop=(j == 1))
                        a2 = msb.tile([128, 512], bf16, tag="a2")
                        nc.scalar.activation(
                            a2[:, 0:gn], o2[:, 0:gn], AF.Prelu,
                            alpha=NEG_MLP, bias=b2_t[:])
                        o3 = mps.tile([16, 512], f32, tag="sm", name="o3_t")
                        nc.tensor.matmul(o3[0:10, 0:gn], W3_t[:],
                                         a2[:, 0:gn])
                        z = msb.tile([16, 512], bf16, tag="z")
                        nc.scalar.activation(
                            z[0:10, 0:gn], o3[0:10, 0:gn], AF.Prelu,
                            alpha=NEG_MLP, bias=b3_t[0:10, :])
                        nc.scalar.activation(z[0:10, 0:gn], z[0:10, 0:gn], AF.Exp)
                        ssum = mps.tile([16, 512], f32, tag="sm", name="ssum_t")[0:1, :]
                        nc.tensor.matmul(
                            ssum[:, 0:gn], ones_t[0:10, 0:1],
                            z[0:10, 0:gn])
                        sinv = msb.tile([1, 512], bf16, tag="sinv")
                        with nc.allow_low_precision(reason="softmax denom bf16"):
                            nc.vector.reciprocal(sinv[:, 0:gn], ssum[:, 0:gn])
                        sx = mps.tile([16, 512], f32, tag="sm", name="sx_t")
                        nc.tensor.matmul(
                            sx[0:10, 0:gn], ones_t[0:1, 0:10],
                            sinv[:, 0:gn])
                        res = msb.tile([16, 512], f32, tag="res")
                        nc.vector.tensor_mul(
                            res[0:10, 0:gn], z[0:10, 0:gn], sx[0:10, 0:gn])
                        nc.sync.dma_start(
                            outT[:, g0 * NT:g0 * NT + gn], res[0:10, 0:gn])

    nc.compile()
    return nc


def _inputs_per_core(inputs, src_w, ad_w, dst_col, meta):
    x = np.asarray(inputs["x"], dtype=np.float32)
    fb = np.asarray(inputs["fb"], dtype=np.float32)
    Wg = np.asarray(inputs["Wg"], dtype=np.float32)
    bias_g = np.asarray(inputs["bias_g"], dtype=np.float32)
    att_src = np.asarray(inputs["att_src"], dtype=np.float32)
    att_dst = np.asarray(inputs["att_dst"], dtype=np.float32)
    W1 = np.asarray(inputs["W1"], dtype=np.float32)
    b1 = np.asarray(inputs["b1"], dtype=np.float32)
    W2 = np.asarray(inputs["W2"], dtype=np.float32)
    b2 = np.asarray(inputs["b2"], dtype=np.float32)
    W3 = np.asarray(inputs["W3"], dtype=np.float32)
    b3 = np.asarray(inputs["b3"], dtype=np.float32)

    x_pad = np.zeros((NPAD, NFP), dtype=np.float32)
    x_pad[:N, :NF] = x
    fb_pad = np.zeros((NFP, NMEL), dtype=np.float32)
    fb_pad[:NF] = fb

    att_blk_s = np.zeros((HC, 4), dtype=np.float32)
    att_blk_d = np.zeros((HC, 4), dtype=np.float32)
    for h in range(H):
        att_blk_s[h * C:(h + 1) * C, h] = att_src[h]
        att_blk_d[h * C:(h + 1) * C, h] = att_dst[h]

    b1p = np.zeros((128, 2), dtype=np.float32)
    b1p[:, 0] = b1[:128]
    b1p[:, 1] = b1[128:]
    b2p = b2.reshape(128, 1).astype(np.float32)
    b3p = np.zeros((128, 1), dtype=np.float32)
    b3p[:10, 0] = b3

    iota_f32 = np.tile(np.arange(128, dtype=np.float32)[None, :], (128, 1))
    common = {
        "fb_p": _to_bf16(fb_pad), "Wg": Wg, "Wg_bf": _to_bf16(Wg),
        "attb_s": att_blk_s, "attb_d": att_blk_d,
        "bias_bc": np.tile(bias_g[None, :], (128, 1)).astype(np.float32),
        "W1": _to_bf16(W1), "b1": b1p, "W2": _to_bf16(W2), "b2": b2p,
        "W3": _to_bf16(W3), "b3": b3p,
        "eye": np.eye(128, dtype=np.float32),
        "eyebf": _to_bf16(np.eye(128, dtype=np.float32)),
        "iota": _to_bf16(iota_f32),
        "ones": _to_bf16(np.ones((128, 16), dtype=np.float32)),
    }

    maps = []
    for k in range(NCORES):
        m = dict(common)
        m["xT_in"] = _to_bf16(
            np.ascontiguousarray(x_pad[k * NPC:(k + 1) * NPC].T))
        m["idx_src"] = src_w[k]
        m["idx_ad"] = ad_w[k]
        m["dst_col"] = dst_col[k].astype(np.float32)
        maps.append(m)
    return maps


def kernel(**inputs):
    from concourse.bass_utils import run_bass_kernel_spmd

    src_w, ad_w, dst_col, meta = _prep(inputs["edge_index"])
    key = ("nc", meta["TOTC"], tuple(meta["cpt"].reshape(-1)),
           tuple(meta["woff"]))
    if key not in _CACHE:
        _CACHE.clear()
        _CACHE[key] = _build(meta)
    nc = _CACHE[key]
    maps = _inputs_per_core(inputs, src_w, ad_w, dst_col, meta)
    res = run_bass_kernel_spmd(nc, maps, core_ids=list(range(NCORES)))
    out = np.zeros((NPAD, 10), dtype=np.float32)
    for k in range(NCORES):
        out[k * NPC:(k + 1) * NPC] = res.results[k]["outT"].T
    return out[:N]
